# revision 4
# baseline (speedup 1.0000x reference)
"""Trainium2 Bass kernel for nn_Blur: depthwise 4x4 separable blur.

Reference semantics:
  h: (8, 256, 64, 512) f32
  pad W circular by 1, pad H reflect by 1, depthwise conv with
  outer([1,3,3,1],[1,3,3,1])/64, VALID -> out (8, 256, 63, 511).

Strategy (fp8 DoubleRow dual-stream; PE does all conv work):
  - Batch-parallel: core i processes h[i] (256, 64, 512).
  - Input is sent as TWO e4m3 streams: x8 = e4m3(32*h) and the
    quantization residual r8 = e4m3(512*(h - x8/32)).  x8 + r8/16
    reconstructs 32*h to ~1e-3 relative accuracy, so the fp8 path
    keeps l2 error ~2e-3 (gate is 2e-2) while halving DMA traffic
    and enabling the fp8 DoubleRow matmul (0.5 cycles/output col).
  - Host packs, per supertile (4 channel-pairs), rows of 8 chunks
    [x8_pair0 | r8_pair0 | x8_pair1 | ...], each chunk 528 cols:
    [w511 | w0..w511 | w0 w1 | 13 pad] with the circular wrap baked
    in.  528 % 16 == 0 satisfies the dual-fp8 ISA restriction that
    the DoubleRow k-subtile stride be a multiple of 16; rows are
    4224 B contiguous (64B-aligned), one descriptor per partition.
  - The H-conv (reflect pad, taps [1,3,3,1]) is the 126x128
    block-diagonal matrix A folded into the stationary operand.
    Per tap dx, ONE DoubleRow matmul computes
      kx[dx]*A @ x8_window(dx)  +  (kx[dx]/16)*A @ r8_window(dx)
    (the two k-subtiles of the dual-fp8 mode), accumulating 4 taps
    into PSUM.  PSUM = 2048 * out_true.
  - PSUM -> SBUF bf16 converts (scale 2^-11) are split across
    ACT / DVE / Pool by column ranges; output rows are 4096 B
    contiguous bf16, one 2D DMA per supertile on the ACT ring.
  - Host de-permutes, strips the 512th col, upcasts to f32.
"""

import numpy as np
import ml_dtypes

import bass_rust
import concourse.bacc as bacc
import concourse.mybir as mybir
from concourse.tile import TileContext
from concourse.bass_utils import run_bass_kernel_spmd

B, C, H, W = 8, 256, 64, 512
HO, WO = H - 1, W - 1  # 63, 511
N_CORES = 8
NPAIR = C // 2           # 128 channel-pairs per core
NSUP = NPAIR // 4        # 32 supertiles (4 pairs each)
CHUNK = 528              # [wrap | 512 | wrap wrap | pad13]; % 16 == 0
PAIRW = 2 * CHUNK        # x-chunk + r-chunk
ROWW = 4 * PAIRW         # 4224 fp8 bytes per input row
OUTW = 4 * W             # 2048 bf16 cols per output row
S_X = 32.0               # x8 = e4m3(32*h);  |32*h| < 240 (IEEE e4m3 max)
S_R = 512.0              # r8 = e4m3(512*(h - x8/32))
PSUM_SCALE = 1.0 / 2048.0  # PSUM = 64*32*out_true
# converts: column split across ACT / DVE (GPSIMD cannot read PSUM)
ACT_COLS = 1120
DVE_COLS = OUTW - ACT_COLS

KX = [1.0, 3.0, 3.0, 1.0]


def _h_matrix():
    """A2p [128, 128]: rows 0:126 = block-diag H-conv (taps [1,3,3,1],
    reflect pad, NO normalization), rows 126:128 zero."""
    k = KX
    A = np.zeros((HO, H), dtype=np.float64)
    for i in range(HO):
        for dy in range(4):
            j = i + dy  # index into reflect-padded H (0..65)
            m = 1 if j == 0 else (H - 2 if j == H + 1 else j - 1)
            A[i, m] += k[dy]
    A2p = np.zeros((128, 128), dtype=np.float64)
    A2p[:HO, :H] = A
    A2p[HO : 2 * HO, H:] = A
    return A2p


def _weights():
    """w [128, 8, 128] e4m3: plane (2*dx) = (kx[dx]*A2p)^T for the x
    stream, plane (2*dx+1) = (kx[dx]/16 * A2p)^T for the residual
    stream.  All values are exactly representable in e4m3."""
    A2p = _h_matrix()
    w = np.zeros((128, 8, 128), dtype=np.float64)
    for dx in range(4):
        w[:, 2 * dx, :] = (KX[dx] * A2p).T
        w[:, 2 * dx + 1, :] = (KX[dx] / 16.0 * A2p).T
    wq = w.astype(ml_dtypes.float8_e4m3)
    assert np.all(wq.astype(np.float64) == w), "weights must be e4m3-exact"
    return wq


def _build_nc(repeat=1):
    """repeat>1 re-runs the per-core workload inside a hardware loop
    (writing the same outputs) -- used only for slope-based timing."""
    nc = bacc.Bacc()
    d = nc.declare_dram_parameter("d", [NSUP * 128, ROWW], mybir.dt.float8e4, isOutput=False)
    w = nc.declare_dram_parameter("w", [128, 8, 128], mybir.dt.float8e4, isOutput=False)
    out = nc.declare_dram_parameter("out", [NSUP * 126, OUTW], mybir.dt.bfloat16, isOutput=True)

    with TileContext(nc) as tc:
        with (
            tc.tile_pool(name="wpool", bufs=1) as wpool,
            tc.tile_pool(name="inpool", bufs=4) as inpool,
            tc.tile_pool(name="psum", bufs=2, space="PSUM") as psum,
            tc.tile_pool(name="outpool", bufs=4) as outpool,
        ):
            w_t = wpool.tile([128, 8, 128], mybir.dt.float8e4, name="w_t")
            nc.sync.dma_start(out=w_t[:, :, :], in_=w[:, :, :])

            def one_pass():
                for st in range(NSUP):
                    t = inpool.tile([128, ROWW], mybir.dt.float8e4, name="t", tag="t")
                    nc.sync.dma_start(
                        out=t[:], in_=d[128 * st : 128 * (st + 1), :]
                    )
                    p = psum.tile([128, OUTW], mybir.dt.float32, name="p", tag="p")
                    for j in range(4):
                        for dx in range(4):
                            base = t[:, j * PAIRW + dx :]
                            rhs = bass_rust.AP(
                                base.tensor,
                                base.offset,
                                [[ROWW, 128], [CHUNK, 2], [1, W]],
                            )
                            nc.tensor.matmul(
                                p[:, W * j : W * (j + 1)],
                                lhsT=w_t[:, 2 * dx : 2 * dx + 2, :],
                                rhs=rhs,
                                start=(dx == 0),
                                stop=(dx == 3),
                                perf_mode=mybir.MatmulPerfMode.DoubleRow,
                            )
                    o = outpool.tile([128, OUTW], mybir.dt.bfloat16, name="o", tag="o")
                    c0 = ACT_COLS
                    nc.scalar.mul(o[:, 0:c0], p[:, 0:c0], PSUM_SCALE)
                    nc.vector.tensor_scalar_mul(o[:, c0:OUTW], p[:, c0:OUTW], PSUM_SCALE)
                    nc.scalar.dma_start(
                        out=out[126 * st : 126 * (st + 1), :], in_=o[0:126, :]
                    )

            if repeat > 1:
                with tc.For_i(0, repeat, 1):
                    one_pass()
            else:
                one_pass()
    if not nc.is_finalized():
        nc.finalize()
    return nc


_NC_CACHE = None


def _get_nc():
    global _NC_CACHE
    if _NC_CACHE is None:
        _NC_CACHE = _build_nc()
    return _NC_CACHE


def _quantize(h):
    """h (B, C, H, W) f32 -> x8, r8 e4m3 arrays of the same shape."""
    hf = np.asarray(h, np.float32)
    x8 = (hf * S_X).astype(ml_dtypes.float8_e4m3)
    r = hf - x8.astype(np.float32) / S_X
    r8 = (r * S_R).astype(ml_dtypes.float8_e4m3)
    return x8, r8


def _pack_core(x8c, r8c):
    """x8c, r8c (C, H, W) e4m3 -> D [NSUP*128, ROWW] e4m3.
    Row (st*128 + p), chunk (j, s): stream s of pair 4*st+j, channel
    2*cp + p//64, H-row p%64, cols [w511 | w0..511 | w0 w1 | pad]."""
    def chunks(a):
        # (C,H,W) -> (NSUP, 2, 64, 4, 528): st, chin, h, j, padded cols
        a = a.reshape(NSUP, 4, 2, H, W).transpose(0, 2, 3, 1, 4)
        pad = np.zeros(a.shape[:-1] + (13,), dtype=a.dtype)
        return np.concatenate(
            [a[..., W - 1 : W], a, a[..., 0:2], pad], axis=-1
        )
    cx = chunks(x8c)  # (NSUP, 2, 64, 4, 528)
    cr = chunks(r8c)
    both = np.stack([cx, cr], axis=4)  # (NSUP, 2, 64, 4, 2, 528)
    return np.ascontiguousarray(both.reshape(NSUP * 128, ROWW))


def _unpack_core(o):
    """o [NSUP*126, OUTW] bf16 -> (C, HO, WO) f32."""
    o = o.reshape(NSUP, 2, HO, 4, W).transpose(0, 3, 1, 2, 4)
    return o.reshape(C, HO, W)[:, :, :WO].astype(np.float32)


def _shard_inputs(h):
    x8, r8 = _quantize(h)
    w = _weights()
    return [
        {"d": _pack_core(x8[i], r8[i]), "w": w} for i in range(N_CORES)
    ]


def kernel(h, _trace=False):
    assert h.shape == (B, C, H, W)
    in_maps = _shard_inputs(h)
    nc = _get_nc()
    res = run_bass_kernel_spmd(nc, in_maps, list(range(N_CORES)), trace=_trace)
    out = np.stack(
        [_unpack_core(res.results[i]["out"]) for i in range(N_CORES)], axis=0
    )
    out = np.ascontiguousarray(out)
    if _trace:
        return out, res
    return out


# revision 10
# speedup vs baseline: 3.0763x; 3.0763x over previous
"""Trainium2 Bass kernel for nn_Blur: depthwise 4x4 separable blur.

Reference semantics:
  h: (8, 256, 64, 512) f32
  pad W circular by 1, pad H reflect by 1, depthwise conv with
  outer([1,3,3,1],[1,3,3,1])/64, VALID -> out (8, 256, 63, 511).

Strategy (fp8 DoubleRow dual-stream; PE does all conv work):
  - Batch-parallel: core i processes h[i] (256, 64, 512).
  - Input is sent as TWO e4m3 streams: x8 = e4m3(32*h) and the
    quantization residual r8 = e4m3(512*(h - x8/32)).  x8 + r8/16
    reconstructs 32*h to ~1e-3 relative accuracy, so the fp8 path
    keeps l2 error ~2e-3 (gate is 2e-2) while halving DMA traffic
    and enabling the fp8 DoubleRow matmul (0.5 cycles/output col).
  - Host packs, per supertile (4 channel-pairs), rows of 8 chunks
    [x8_pair0 | r8_pair0 | x8_pair1 | ...], each chunk 528 cols:
    [w511 | w0..w511 | w0 w1 | 13 pad] with the circular wrap baked
    in.  528 % 16 == 0 satisfies the dual-fp8 ISA restriction that
    the DoubleRow k-subtile stride be a multiple of 16; rows are
    4224 B contiguous (64B-aligned), one descriptor per partition.
  - The H-conv (reflect pad, taps [1,3,3,1]) is the 126x128
    block-diagonal matrix A folded into the stationary operand.
    Per tap dx, ONE DoubleRow matmul computes
      kx[dx]*A @ x8_window(dx)  +  (kx[dx]/16)*A @ r8_window(dx)
    (the two k-subtiles of the dual-fp8 mode), accumulating 4 taps
    into PSUM.  PSUM = 2048 * out_true.
  - PSUM -> SBUF bf16 converts (scale 2^-11) are split across
    ACT / DVE / Pool by column ranges; output rows are 4096 B
    contiguous bf16, one 2D DMA per supertile on the ACT ring.
  - Host de-permutes, strips the 512th col, upcasts to f32.
"""

import numpy as np
import ml_dtypes

import bass_rust
import concourse.bacc as bacc
import concourse.mybir as mybir
from concourse.tile import TileContext
from concourse.bass_utils import run_bass_kernel_spmd

B, C, H, W = 8, 256, 64, 512
HO, WO = H - 1, W - 1  # 63, 511
N_CORES = 8
NPAIR = C // 2           # 128 channel-pairs per core
NSUP = NPAIR // 4        # 32 supertiles (4 pairs each)
CHUNK = 528              # [wrap | 512 | wrap wrap | pad13]; % 16 == 0
PAIRW = 2 * CHUNK        # x-chunk + r-chunk
ROWW = 4 * PAIRW         # 4224 fp8 bytes per input row
OUTW = 4 * W             # 2048 bf16 cols per output row
S_X = 32.0               # x8 = e4m3(32*h);  |32*h| < 240 (IEEE e4m3 max)
S_R = 512.0              # r8 = e4m3(512*(h - x8/32))
PSUM_SCALE = 1.0 / 2048.0  # PSUM = 64*32*out_true
# converts: column split across ACT / DVE (GPSIMD cannot read PSUM)
ACT_COLS = 1120
DVE_COLS = OUTW - ACT_COLS

KX = [1.0, 3.0, 3.0, 1.0]


def _h_matrix():
    """A2p [128, 128]: rows 0:126 = block-diag H-conv (taps [1,3,3,1],
    reflect pad, NO normalization), rows 126:128 zero."""
    k = KX
    A = np.zeros((HO, H), dtype=np.float64)
    for i in range(HO):
        for dy in range(4):
            j = i + dy  # index into reflect-padded H (0..65)
            m = 1 if j == 0 else (H - 2 if j == H + 1 else j - 1)
            A[i, m] += k[dy]
    A2p = np.zeros((128, 128), dtype=np.float64)
    A2p[:HO, :H] = A
    A2p[HO : 2 * HO, H:] = A
    return A2p


def _weights():
    """w [128, 8, 128] e4m3: plane (2*dx) = (kx[dx]*A2p)^T for the x
    stream, plane (2*dx+1) = (kx[dx]/16 * A2p)^T for the residual
    stream.  All values are exactly representable in e4m3."""
    A2p = _h_matrix()
    w = np.zeros((128, 8, 128), dtype=np.float64)
    for dx in range(4):
        w[:, 2 * dx, :] = (KX[dx] * A2p).T
        w[:, 2 * dx + 1, :] = (KX[dx] / 16.0 * A2p).T
    wq = w.astype(ml_dtypes.float8_e4m3)
    assert np.all(wq.astype(np.float64) == w), "weights must be e4m3-exact"
    return wq


def _build_nc(repeat=1, dma_only=False):
    """repeat>1 re-runs the per-core workload inside a hardware loop
    (writing the same outputs) -- used only for slope-based timing.
    dma_only=True keeps just the DMA pattern (no matmuls/converts) to
    measure the memory-system floor."""
    nc = bacc.Bacc()
    d = nc.declare_dram_parameter("d", [NSUP * 128, ROWW], mybir.dt.float8e4, isOutput=False)
    w = nc.declare_dram_parameter("w", [128, 8, 128], mybir.dt.float8e4, isOutput=False)
    out = nc.declare_dram_parameter("out", [NSUP * 126, OUTW], mybir.dt.bfloat16, isOutput=True)

    GS = 4                 # supertiles per group (DMA granularity)
    NG = NSUP // GS        # 8 groups
    GIN = GS * ROWW        # 16896 input cols per group tile
    GOUT = GS * OUTW       # 8192 output cols per group tile

    with TileContext(nc) as tc:
        with (
            tc.tile_pool(name="wpool", bufs=1) as wpool,
            tc.tile_pool(name="inpool", bufs=2) as inpool,
            tc.tile_pool(name="psum", bufs=2, space="PSUM") as psum,
            tc.tile_pool(name="outpool", bufs=2) as outpool,
        ):
            w_t = wpool.tile([128, 8, 128], mybir.dt.float8e4, name="w_t")
            nc.sync.dma_start(out=w_t[:, :, :], in_=w[:, :, :])

            def one_pass():
                for g in range(NG):
                    t = inpool.tile([128, GIN], mybir.dt.float8e4, name="t", tag="t")
                    src = d[128 * GS * g :]
                    nc.sync.dma_start(
                        out=bass_rust.AP(
                            t[:].tensor,
                            t[:].offset,
                            [[GIN, 128], [ROWW, GS], [1, ROWW]],
                        ),
                        in_=bass_rust.AP(
                            src.tensor,
                            src.offset,
                            [[ROWW, 128], [128 * ROWW, GS], [1, ROWW]],
                        ),
                    )
                    o = outpool.tile([128, GOUT], mybir.dt.bfloat16, name="o", tag="o")
                    for jj in range(GS) if not dma_only else []:
                        p = psum.tile([128, OUTW], mybir.dt.float32, name="p", tag="p")
                        for j in range(4):
                            for dx in range(4):
                                base = t[:, jj * ROWW + j * PAIRW + dx :]
                                rhs = bass_rust.AP(
                                    base.tensor,
                                    base.offset,
                                    [[GIN, 128], [CHUNK, 2], [1, W]],
                                )
                                nc.tensor.matmul(
                                    p[:, W * j : W * (j + 1)],
                                    lhsT=w_t[:, 2 * dx : 2 * dx + 2, :],
                                    rhs=rhs,
                                    start=(dx == 0),
                                    stop=(dx == 3),
                                    perf_mode=mybir.MatmulPerfMode.DoubleRow,
                                )
                        c0 = ACT_COLS
                        ob = jj * OUTW
                        nc.scalar.mul(o[:, ob : ob + c0], p[:, 0:c0], PSUM_SCALE)
                        nc.vector.tensor_scalar_mul(
                            o[:, ob + c0 : ob + OUTW], p[:, c0:OUTW], PSUM_SCALE
                        )
                    dst = out[126 * GS * g :]
                    if dma_only:
                        tb = t[:].bitcast(mybir.dt.bfloat16)
                        src_ap = bass_rust.AP(
                            tb.tensor, tb.offset,
                            [[GIN // 2, 126], [OUTW, GS], [1, OUTW]],
                        )
                    else:
                        src_ap = bass_rust.AP(
                            o[:].tensor, o[:].offset,
                            [[GOUT, 126], [OUTW, GS], [1, OUTW]],
                        )
                    nc.gpsimd.dma_start(
                        out=bass_rust.AP(
                            dst.tensor,
                            dst.offset,
                            [[OUTW, 126], [126 * OUTW, GS], [1, OUTW]],
                        ),
                        in_=src_ap,
                    )

            if repeat > 1:
                with tc.For_i(0, repeat, 1):
                    one_pass()
            else:
                one_pass()
    if not nc.is_finalized():
        nc.finalize()
    return nc


_NC_CACHE = None


def _get_nc():
    global _NC_CACHE
    if _NC_CACHE is None:
        _NC_CACHE = _build_nc()
    return _NC_CACHE


def _quantize(h):
    """h (B, C, H, W) f32 -> x8, r8 e4m3 arrays of the same shape."""
    hf = np.asarray(h, np.float32)
    x8 = (hf * S_X).astype(ml_dtypes.float8_e4m3)
    r = hf - x8.astype(np.float32) / S_X
    r8 = (r * S_R).astype(ml_dtypes.float8_e4m3)
    return x8, r8


def _pack_core(x8c, r8c):
    """x8c, r8c (C, H, W) e4m3 -> D [NSUP*128, ROWW] e4m3.
    Row (st*128 + p), chunk (j, s): stream s of pair 4*st+j, channel
    2*cp + p//64, H-row p%64, cols [w511 | w0..511 | w0 w1 | pad]."""
    def chunks(a):
        # (C,H,W) -> (NSUP, 2, 64, 4, 528): st, chin, h, j, padded cols
        a = a.reshape(NSUP, 4, 2, H, W).transpose(0, 2, 3, 1, 4)
        pad = np.zeros(a.shape[:-1] + (13,), dtype=a.dtype)
        return np.concatenate(
            [a[..., W - 1 : W], a, a[..., 0:2], pad], axis=-1
        )
    cx = chunks(x8c)  # (NSUP, 2, 64, 4, 528)
    cr = chunks(r8c)
    both = np.stack([cx, cr], axis=4)  # (NSUP, 2, 64, 4, 2, 528)
    return np.ascontiguousarray(both.reshape(NSUP * 128, ROWW))


def _unpack_core(o):
    """o [NSUP*126, OUTW] bf16 -> (C, HO, WO) f32."""
    o = o.reshape(NSUP, 2, HO, 4, W).transpose(0, 3, 1, 2, 4)
    return o.reshape(C, HO, W)[:, :, :WO].astype(np.float32)


def _shard_inputs(h):
    x8, r8 = _quantize(h)
    w = _weights()
    return [
        {"d": _pack_core(x8[i], r8[i]), "w": w} for i in range(N_CORES)
    ]


def kernel(h, _trace=False):
    assert h.shape == (B, C, H, W)
    in_maps = _shard_inputs(h)
    nc = _get_nc()
    res = run_bass_kernel_spmd(nc, in_maps, list(range(N_CORES)), trace=_trace)
    out = np.stack(
        [_unpack_core(res.results[i]["out"]) for i in range(N_CORES)], axis=0
    )
    out = np.ascontiguousarray(out)
    if _trace:
        return out, res
    return out
